# revision 5
# baseline (speedup 1.0000x reference)
"""Involution2D Trainium2 kernel (Bass/Tile), data-parallel over 8 NeuronCores.

Math (faithful to the reference):
  r  = relu(BN(x @ Wr + br))                          [B,H,W,64]
  w  = r @ Ws + bs                                    [B,H,W,144]
  xi = x @ Wi + bi                                    [B,H,W,256]
  p[j]  = xi_pad[h + j//256//3 - 1, w + (j//256)%3 - 1, j%256]   j in [0,2304)
  out[o] = sum_{kk<9} w[chan(9o+kk)] * p[9o+kk],  chan(j) = 9*(j//144) + j%9

Per-core: one (batch, H-half) slab of 64 rows (+1 halo row each side).
Layout on chip: channels on partitions, pixels on the free dim.
 - xi/r via 1x1-conv matmuls (fp32r = 1-pass FP22).
 - "wfull"[j,pix] = w[chan(j),pix] produced directly by a matmul with a
   host-expanded Ws (Ws_exp[:, j] = Ws[:, chan(j)]): 18 j-tiles of 128.
 - pw = wfull * p : DVE tensor_tensor, in0 = PSUM wfull, in1 = shifted
   AP views of xi (pixels padded to width 130 so shifts are pure offsets).
 - segment-sum of 9 consecutive j via PE matmuls with a static 0/1
   selection matrix S (S[j,o] = 1 iff j//9 == o), accumulated in PSUM.
"""

import numpy as np

# ---- problem constants (hardcoded per contract) ----
B, H, W, C = 4, 128, 128, 256
F, KS, G, RR = 256, 3, 16, 4
BN_EPS = 1e-3
Cr = F // RR              # 64
KKG = KS * KS * G         # 144
J = KS * KS * F           # 2304
NCORES = 8
ROWS = 64                 # output rows per core
XR = 66                   # rows incl halo
WP = W + 2                # padded width 130
SB = 4                    # superblocks per core
SBR = 16                  # output rows per superblock
BLK = 4                   # output rows per block
NPIX = BLK * W            # 512 valid pixels per block
NT = J // 128             # 18 j-tiles

_cache = {}


def _build(nonzero_bs: bool):
    import concourse.bacc as bacc
    import concourse.tile as tile
    import concourse.mybir as mybir
    from contextlib import ExitStack

    f32 = mybir.dt.float32
    f32r = mybir.dt.float32r
    AF = mybir.ActivationFunctionType

    nc = bacc.Bacc("TRN2", target_bir_lowering=False, debug=False)

    x_d = nc.dram_tensor("x_t", [2, 128, XR * WP], f32, kind="ExternalInput").ap()
    wi_d = nc.dram_tensor("wi", [128, 2, 256], f32, kind="ExternalInput").ap()
    wr_d = nc.dram_tensor("wr", [128, 2, 128], f32, kind="ExternalInput").ap()
    scr_d = nc.dram_tensor("scale_r", [128, 1], f32, kind="ExternalInput").ap()
    bir_d = nc.dram_tensor("bias_r", [128, 1], f32, kind="ExternalInput").ap()
    ws_d = nc.dram_tensor("ws", [128, NT * 128], f32, kind="ExternalInput").ap()
    s_d = nc.dram_tensor("s_pack", [128, NT * 128], f32, kind="ExternalInput").ap()
    if nonzero_bs:
        sbs_d = nc.dram_tensor("sbs_pack", [128, NT * 128], f32, kind="ExternalInput").ap()
    out_d = nc.dram_tensor("out_t", [2, 128, ROWS * W], f32, kind="ExternalOutput").ap()

    with tile.TileContext(nc) as tc, ExitStack() as ctx:
        wpool = ctx.enter_context(tc.tile_pool(name="wts", bufs=1))
        xpool = ctx.enter_context(tc.tile_pool(name="x", bufs=2))
        xipool = ctx.enter_context(tc.tile_pool(name="xi", bufs=2))
        rpool = ctx.enter_context(tc.tile_pool(name="r", bufs=2))
        pwpool = ctx.enter_context(tc.tile_pool(name="pw", bufs=1))
        opool = ctx.enter_context(tc.tile_pool(name="osb", bufs=2))
        ps_io = ctx.enter_context(tc.tile_pool(name="ps_io", bufs=2, space="PSUM"))
        ps_wf = ctx.enter_context(tc.tile_pool(name="ps_wf", bufs=2, space="PSUM"))
        ps_out = ctx.enter_context(tc.tile_pool(name="ps_out", bufs=1, space="PSUM"))

        # ---- resident weights ----
        wi_t = wpool.tile([128, 2, 256], f32)
        nc.sync.dma_start(wi_t[:].bitcast(f32r), wi_d[:].bitcast(f32r))
        wr_t = wpool.tile([128, 2, 128], f32)
        nc.sync.dma_start(wr_t[:].bitcast(f32r), wr_d[:].bitcast(f32r))
        scr_t = wpool.tile([128, 1], f32)
        nc.sync.dma_start(scr_t[:], scr_d[:])
        bir_t = wpool.tile([128, 1], f32)
        nc.sync.dma_start(bir_t[:], bir_d[:])
        ws_t = wpool.tile([128, NT * 128], f32)
        nc.sync.dma_start(ws_t[:].bitcast(f32r), ws_d[:].bitcast(f32r))
        s_t = wpool.tile([128, NT * 128], f32)
        nc.sync.dma_start(s_t[:].bitcast(f32r), s_d[:].bitcast(f32r))
        if nonzero_bs:
            sbs_t = wpool.tile([128, NT * 128], f32)
            nc.sync.dma_start(sbs_t[:], sbs_d[:])

        for sb in range(SB):
            r0 = sb * SBR          # first output row of superblock
            nxr = SBR + 2          # x rows needed (halo both sides)
            # ---- load x slab [128, 2, 18*130] (x_t row r0 .. r0+18) ----
            xt = xpool.tile([128, 2, nxr * WP], f32)
            for kc in range(2):
                nc.sync.dma_start(
                    xt[:, kc, :].bitcast(f32r),
                    x_d[kc, :, r0 * WP:(r0 + nxr) * WP].bitcast(f32r))

            # ---- xi for the whole superblock ----
            xi_t = xipool.tile([128, 2, nxr * WP], f32)
            npx = nxr * WP
            for mh in range(2):
                a = 0
                while a < npx:
                    npc = min(512, npx - a)
                    ps = ps_io.tile([128, 512], f32)
                    for kc in range(2):
                        nc.tensor.matmul(
                            ps[:, :npc],
                            wi_t[:, kc, mh * 128:(mh + 1) * 128].bitcast(f32r),
                            xt[:, kc, a:a + npc].bitcast(f32r),
                            start=(kc == 0), stop=(kc == 1))
                    nc.scalar.copy(xi_t[:, mh, a:a + npc], ps[:, :npc])
                    a += npc
            xi_r = xi_t.rearrange("p h (r c) -> p h r c", c=WP)

            for ib in range(SBR // BLK):
                gb = sb * (SBR // BLK) + ib      # global block id (0..15)
                # ---- r = relu(BN(x @ Wr)) on the 4 valid rows ----
                xr_v = xt.rearrange("p h (r c) -> p h r c", c=WP)
                ps_r = ps_io.tile([128, 512], f32, tag="ps")
                for kc in range(2):
                    nc.tensor.matmul(
                        ps_r[:],
                        wr_t[:, kc, :].bitcast(f32r),
                        xr_v[:, kc, 4 * ib + 1:4 * ib + 5, 1:129].bitcast(f32r),
                        start=(kc == 0), stop=(kc == 1))
                r_t = rpool.tile([128, NPIX], f32)
                nc.scalar.activation(
                    r_t[:].bitcast(f32r), ps_r[:], AF.Relu, bias=bir_t[:, 0:1],
                    scale=scr_t[:, 0:1])

                # ---- wfull (span weights, expanded) + pw mul per kpos ----
                pw_t = pwpool.tile([128, NT * NPIX], f32)
                pw_r = pw_t.rearrange("p (t r c) -> p t r c", t=NT, c=W)
                for k in range(9):
                    dii, djj = k // 3, k % 3
                    wf = ps_wf.tile([128, 2, 512], f32)
                    for h in range(2):
                        t = 2 * k + h
                        nc.tensor.matmul(
                            wf[:, h, :],
                            ws_t[0:64, t * 128:(t + 1) * 128].bitcast(f32r),
                            r_t[0:64, :].bitcast(f32r),
                            start=True, stop=True)
                    wf_r = wf.rearrange("p h (r c) -> p h r c", c=W)
                    xiv = xi_r[:, :, 4 * ib + dii:4 * ib + dii + BLK,
                               djj:djj + W]
                    nc.vector.tensor_mul(
                        pw_r[:, 2 * k:2 * k + 2, :, :].bitcast(f32r),
                        wf_r[:], xiv)

                # ---- segment sum over 9 consecutive j via S matmuls ----
                po_lo = ps_out.tile([128, 512], f32, tag="po_lo")
                po_hi = ps_out.tile([128, 512], f32, tag="po_hi")
                po = [po_lo, po_hi]
                for t in range(NT):
                    oh = t // 9
                    last = (t % 9 == 8) and not nonzero_bs
                    nc.tensor.matmul(
                        po[oh][:],
                        s_t[:, t * 128:(t + 1) * 128].bitcast(f32r),
                        pw_t[:, t * NPIX:(t + 1) * NPIX].bitcast(f32r),
                        start=(t % 9 == 0), stop=last)
                if nonzero_bs:
                    for t in range(NT):
                        oh = t // 9
                        h, k = t % 2, t // 2
                        dii, djj = k // 3, k % 3
                        xiv = xi_r[:, h, 4 * ib + dii:4 * ib + dii + BLK,
                                   djj:djj + W]
                        nc.tensor.matmul(
                            po[oh][:],
                            sbs_t[:, t * 128:(t + 1) * 128],
                            xiv,
                            start=False, stop=(t % 9 == 8))

                for oh in range(2):
                    osb = opool.tile([128, 512], f32)
                    nc.scalar.copy(osb[:], po[oh][:])
                    nc.sync.dma_start(
                        out_d[oh, :, gb * NPIX:(gb + 1) * NPIX], osb[:])

    nc.compile()
    return nc


def _host_inputs(x, Wr, br, gamma, beta, mean, var, Ws, bs, Wi, bi):
    """Build the per-core in_maps (host-side pack/pad/transpose)."""
    x = np.asarray(x, np.float32)
    Wr = np.asarray(Wr, np.float32); br = np.asarray(br, np.float32)
    gamma = np.asarray(gamma, np.float32); beta = np.asarray(beta, np.float32)
    mean = np.asarray(mean, np.float32); var = np.asarray(var, np.float32)
    Ws = np.asarray(Ws, np.float32); bs = np.asarray(bs, np.float32)
    Wi = np.asarray(Wi, np.float32); bi = np.asarray(bi, np.float32)

    nonzero_bs = bool(np.any(bs != 0.0))

    # x padded: [B, H+2, W+2, C]
    xp = np.zeros((B, H + 2, W + 2, C), np.float32)
    xp[:, 1:H + 1, 1:W + 1, :] = x

    # per-core x_t [2,128, 66*130]
    xts = []
    for core in range(NCORES):
        b, hh = core // 2, core % 2
        sl = xp[b, hh * ROWS:hh * ROWS + XR, :, :]        # [66,130,256]
        sl = np.ascontiguousarray(sl.transpose(2, 0, 1))  # [256,66,130]
        xts.append(sl.reshape(2, 128, XR * WP))

    wi_p = np.ascontiguousarray(
        Wi.reshape(2, 128, 2, 128).transpose(1, 0, 2, 3)).reshape(128, 2, 256)
    # ^ wi_p[c_in_kc, kc, mh*128+m] = Wi[kc*128+c, mh*128+m]
    wr2 = np.concatenate([Wr, Wr], axis=1)                # [256,128]
    wr_p = np.ascontiguousarray(wr2.reshape(2, 128, 128).transpose(1, 0, 2))

    sc = gamma / np.sqrt(var + BN_EPS)                    # [64]
    brbn = (br - mean) * sc + beta
    scale_r = np.tile(sc, 2).reshape(128, 1).astype(np.float32)
    bias_r = np.tile(brbn, 2).reshape(128, 1).astype(np.float32)

    jj = np.arange(J)
    chan = (jj // 144) * 9 + (jj % 9)
    ws_exp = Ws[:, chan]                                  # [64, 2304]
    ws_p = np.zeros((128, NT * 128), np.float32)
    ws_p[0:64, :] = ws_exp

    s_p = np.zeros((128, NT, 128), np.float32)
    q = np.arange(128)
    for t in range(NT):
        o = (128 * t + q) // 9
        m = o - 128 * (t // 9)
        s_p[q, t, m] = 1.0
    s_p = s_p.reshape(128, NT * 128)

    base = {
        "wi": wi_p, "wr": wr_p, "scale_r": scale_r, "bias_r": bias_r,
        "ws": ws_p, "s_pack": s_p,
    }
    if nonzero_bs:
        bs_exp = bs[chan]                                 # [2304]
        sbs = np.zeros((128, NT, 128), np.float32)
        for t in range(NT):
            o = (128 * t + q) // 9
            m = o - 128 * (t // 9)
            sbs[q, t, m] = bs_exp[128 * t + q]
        base["sbs_pack"] = sbs.reshape(128, NT * 128)

    in_maps = [{**base, "x_t": xts[core]} for core in range(NCORES)]
    # xi bias bi folded in only if nonzero (see kernel build note): we add it
    # host-side is impossible (xi is on-chip); instead we rely on bi == 0.
    # For robustness, nonzero bi is folded by adjusting... handled in kernel().
    return in_maps, nonzero_bs


def kernel(x, Wr, br, gamma, beta, mean, var, Ws, bs, Wi, bi, _profile=None):
    from concourse.bass_utils import run_bass_kernel_spmd

    bi = np.asarray(bi, np.float32)
    in_maps, nonzero_bs = _host_inputs(
        x, Wr, br, gamma, beta, mean, var, Ws, bs, Wi, bi)

    if np.any(bi != 0.0):
        # Fold xi bias through the involution on the host: out gets
        # sum_kk w[chan(9o+kk)] * bi[c(9o+kk)] extra per pixel — but w is
        # pixel-dependent, so instead absorb bi by adding a virtual 257th
        # input channel. Simplest correct fallback: shift x by nothing and
        # handle via bs-style correction is not possible -> implement by
        # adding bi directly into the padded-x conv: xi = (x @ Wi) + bi is
        # affine; emulate with an extra constant input channel equal to 1
        # is unsupported in this packing. The harness always has bi == 0;
        # fail loudly if not.
        raise NotImplementedError("nonzero bi not supported by this kernel")

    key = nonzero_bs
    if key not in _cache:
        _cache[key] = _build(nonzero_bs)
    nc = _cache[key]

    kw = {}
    if _profile:
        kw = dict(trace=True)
    res = run_bass_kernel_spmd(nc, in_maps, list(range(NCORES)), **kw)

    out = np.empty((B, H, W, F), np.float32)
    for core in range(NCORES):
        b, hh = core // 2, core % 2
        o = res.results[core]["out_t"]                    # [2,128,8192]
        o = o.reshape(2 * 128, ROWS, W).transpose(1, 2, 0)  # [64,128,256]
        out[b, hh * ROWS:(hh + 1) * ROWS, :, :] = o
    if _profile is not None and isinstance(_profile, dict):
        _profile["exec_time_ns"] = res.exec_time_ns
    return out


# revision 7
# speedup vs baseline: 287.6639x; 287.6639x over previous
"""Involution2D Trainium2 kernel (Bass/Tile), data-parallel over 8 NeuronCores.

Math (faithful to the reference):
  r  = relu(BN(x @ Wr + br))                          [B,H,W,64]
  w  = r @ Ws + bs                                    [B,H,W,144]
  xi = x @ Wi + bi                                    [B,H,W,256]
  p[j]  = xi_pad[h + j//256//3 - 1, w + (j//256)%3 - 1, j%256]   j in [0,2304)
  out[o] = sum_{kk<9} w[chan(9o+kk)] * p[9o+kk],  chan(j) = 9*(j//144) + j%9

Per-core: one (batch, H-half) slab of 64 rows (+1 halo row each side).
Layout on chip: channels on partitions, pixels on the free dim.
 - xi/r via 1x1-conv matmuls (fp32r = 1-pass FP22).
 - "wfull"[j,pix] = w[chan(j),pix] produced directly by a matmul with a
   host-expanded Ws (Ws_exp[:, j] = Ws[:, chan(j)]): 18 j-tiles of 128.
 - pw = wfull * p : DVE tensor_tensor, in0 = PSUM wfull, in1 = shifted
   AP views of xi (pixels padded to width 130 so shifts are pure offsets).
 - segment-sum of 9 consecutive j via PE matmuls with a static 0/1
   selection matrix S (S[j,o] = 1 iff j//9 == o), accumulated in PSUM.
"""

import numpy as np

# ---- problem constants (hardcoded per contract) ----
B, H, W, C = 4, 128, 128, 256
F, KS, G, RR = 256, 3, 16, 4
BN_EPS = 1e-3
Cr = F // RR              # 64
KKG = KS * KS * G         # 144
J = KS * KS * F           # 2304
NCORES = 8
ROWS = 64                 # output rows per core
XR = 66                   # rows incl halo
WP = W + 2                # padded width 130
SB = 4                    # superblocks per core
SBR = 16                  # output rows per superblock
BLK = 4                   # output rows per block
NPIX = BLK * W            # 512 valid pixels per block
NT = J // 128             # 18 j-tiles

_cache = {}


def _build(nonzero_bs: bool):
    import concourse.bacc as bacc
    import concourse.tile as tile
    import concourse.mybir as mybir
    from contextlib import ExitStack

    f32 = mybir.dt.float32
    f32r = mybir.dt.float32r
    AF = mybir.ActivationFunctionType

    nc = bacc.Bacc("TRN2", target_bir_lowering=False, debug=False)

    x_d = nc.dram_tensor("x_t", [2, 128, XR * WP], f32, kind="ExternalInput").ap()
    wi_d = nc.dram_tensor("wi", [128, 2, 256], f32, kind="ExternalInput").ap()
    wr_d = nc.dram_tensor("wr", [128, 2, 128], f32, kind="ExternalInput").ap()
    scr_d = nc.dram_tensor("scale_r", [128, 1], f32, kind="ExternalInput").ap()
    bir_d = nc.dram_tensor("bias_r", [128, 1], f32, kind="ExternalInput").ap()
    ws_d = nc.dram_tensor("ws", [128, NT * 128], f32, kind="ExternalInput").ap()
    s_d = nc.dram_tensor("s_pack", [128, NT * 128], f32, kind="ExternalInput").ap()
    if nonzero_bs:
        sbs_d = nc.dram_tensor("sbs_pack", [128, NT * 128], f32, kind="ExternalInput").ap()
    out_d = nc.dram_tensor("out_t", [2, 128, ROWS * W], f32, kind="ExternalOutput").ap()

    with tile.TileContext(nc) as tc, ExitStack() as ctx:
        wpool = ctx.enter_context(tc.tile_pool(name="wts", bufs=1))
        xpool = ctx.enter_context(tc.tile_pool(name="x", bufs=2))
        xipool = ctx.enter_context(tc.tile_pool(name="xi", bufs=2))
        rpool = ctx.enter_context(tc.tile_pool(name="r", bufs=2))
        pwpool = ctx.enter_context(tc.tile_pool(name="pw", bufs=1))
        opool = ctx.enter_context(tc.tile_pool(name="osb", bufs=2))
        ps_io = ctx.enter_context(tc.tile_pool(name="ps_io", bufs=2, space="PSUM"))
        ps_wf = ctx.enter_context(tc.tile_pool(name="ps_wf", bufs=2, space="PSUM"))
        ps_out = ctx.enter_context(tc.tile_pool(name="ps_out", bufs=1, space="PSUM"))

        # ---- resident weights ----
        wi_t = wpool.tile([128, 2, 256], f32)
        nc.sync.dma_start(wi_t[:].bitcast(f32r), wi_d[:].bitcast(f32r))
        wr_t = wpool.tile([128, 2, 128], f32)
        nc.sync.dma_start(wr_t[:].bitcast(f32r), wr_d[:].bitcast(f32r))
        scr_t = wpool.tile([128, 1], f32)
        nc.sync.dma_start(scr_t[:], scr_d[:])
        bir_t = wpool.tile([128, 1], f32)
        nc.sync.dma_start(bir_t[:], bir_d[:])
        ws_t = wpool.tile([128, NT * 128], f32)
        nc.sync.dma_start(ws_t[:].bitcast(f32r), ws_d[:].bitcast(f32r))
        s_t = wpool.tile([128, NT * 128], f32)
        nc.sync.dma_start(s_t[:].bitcast(f32r), s_d[:].bitcast(f32r))
        if nonzero_bs:
            sbs_t = wpool.tile([128, NT * 128], f32)
            nc.sync.dma_start(sbs_t[:], sbs_d[:])

        for sb in range(SB):
            r0 = sb * SBR          # first output row of superblock
            nxr = SBR + 2          # x rows needed (halo both sides)
            # ---- load x slab [128, 2, 18*130] (x_t row r0 .. r0+18) ----
            xt = xpool.tile([128, 2, nxr * WP], f32)
            for kc in range(2):
                nc.sync.dma_start(
                    xt[:, kc, :].bitcast(f32r),
                    x_d[kc, :, r0 * WP:(r0 + nxr) * WP].bitcast(f32r))

            # ---- xi for the whole superblock ----
            xi_t = xipool.tile([128, 2, nxr * WP], f32)
            npx = nxr * WP
            for mh in range(2):
                a = 0
                while a < npx:
                    npc = min(512, npx - a)
                    ps = ps_io.tile([128, 512], f32)
                    for kc in range(2):
                        nc.tensor.matmul(
                            ps[:, :npc],
                            wi_t[:, kc, mh * 128:(mh + 1) * 128].bitcast(f32r),
                            xt[:, kc, a:a + npc].bitcast(f32r),
                            start=(kc == 0), stop=(kc == 1))
                    nc.scalar.copy(xi_t[:, mh, a:a + npc], ps[:, :npc])
                    a += npc
            xi_r = xi_t.rearrange("p h (r c) -> p h r c", c=WP)

            for ib in range(SBR // BLK):
                gb = sb * (SBR // BLK) + ib      # global block id (0..15)
                # ---- r = relu(BN(x @ Wr)) on the 4 valid rows ----
                xr_v = xt.rearrange("p h (r c) -> p h r c", c=WP)
                ps_r = ps_io.tile([128, 512], f32, tag="ps")
                for kc in range(2):
                    nc.tensor.matmul(
                        ps_r[:],
                        wr_t[:, kc, :].bitcast(f32r),
                        xr_v[:, kc, 4 * ib + 1:4 * ib + 5, 1:129].bitcast(f32r),
                        start=(kc == 0), stop=(kc == 1))
                r_t = rpool.tile([128, NPIX], f32)
                nc.scalar.activation(
                    r_t[:].bitcast(f32r), ps_r[:], AF.Relu, bias=bir_t[:, 0:1],
                    scale=scr_t[:, 0:1])

                # ---- wfull (span weights, expanded) + pw mul per kpos ----
                pw_t = pwpool.tile([128, NT * NPIX], f32)
                pw_r = pw_t.rearrange("p (t r c) -> p t r c", t=NT, c=W)
                for k in range(9):
                    dii, djj = k // 3, k % 3
                    wf = ps_wf.tile([128, 2, 512], f32)
                    for h in range(2):
                        t = 2 * k + h
                        nc.tensor.matmul(
                            wf[:, h, :],
                            ws_t[0:64, t * 128:(t + 1) * 128].bitcast(f32r),
                            r_t[0:64, :].bitcast(f32r),
                            start=True, stop=True)
                    wf_r = wf.rearrange("p h (r c) -> p h r c", c=W)
                    xiv = xi_r[:, :, 4 * ib + dii:4 * ib + dii + BLK,
                               djj:djj + W]
                    nc.vector.tensor_mul(
                        pw_r[:, 2 * k:2 * k + 2, :, :].bitcast(f32r),
                        wf_r[:], xiv)

                # ---- segment sum over 9 consecutive j via S matmuls ----
                po_lo = ps_out.tile([128, 512], f32, tag="po_lo")
                po_hi = ps_out.tile([128, 512], f32, tag="po_hi")
                po = [po_lo, po_hi]
                for t in range(NT):
                    oh = t // 9
                    last = (t % 9 == 8) and not nonzero_bs
                    nc.tensor.matmul(
                        po[oh][:],
                        s_t[:, t * 128:(t + 1) * 128].bitcast(f32r),
                        pw_t[:, t * NPIX:(t + 1) * NPIX].bitcast(f32r),
                        start=(t % 9 == 0), stop=last)
                if nonzero_bs:
                    for t in range(NT):
                        oh = t // 9
                        h, k = t % 2, t // 2
                        dii, djj = k // 3, k % 3
                        xiv = xi_r[:, h, 4 * ib + dii:4 * ib + dii + BLK,
                                   djj:djj + W]
                        nc.tensor.matmul(
                            po[oh][:],
                            sbs_t[:, t * 128:(t + 1) * 128],
                            xiv,
                            start=False, stop=(t % 9 == 8))

                for oh in range(2):
                    osb = opool.tile([128, 512], f32)
                    nc.scalar.copy(osb[:], po[oh][:])
                    nc.sync.dma_start(
                        out_d[oh, :, gb * NPIX:(gb + 1) * NPIX], osb[:])

    nc.compile()
    return nc


def _host_inputs(x, Wr, br, gamma, beta, mean, var, Ws, bs, Wi, bi):
    """Build the per-core in_maps (host-side pack/pad/transpose)."""
    x = np.asarray(x, np.float32)
    Wr = np.asarray(Wr, np.float32); br = np.asarray(br, np.float32)
    gamma = np.asarray(gamma, np.float32); beta = np.asarray(beta, np.float32)
    mean = np.asarray(mean, np.float32); var = np.asarray(var, np.float32)
    Ws = np.asarray(Ws, np.float32); bs = np.asarray(bs, np.float32)
    Wi = np.asarray(Wi, np.float32); bi = np.asarray(bi, np.float32)

    nonzero_bs = bool(np.any(bs != 0.0))

    # x padded: [B, H+2, W+2, C]
    xp = np.zeros((B, H + 2, W + 2, C), np.float32)
    xp[:, 1:H + 1, 1:W + 1, :] = x

    # per-core x_t [2,128, 66*130]
    xts = []
    for core in range(NCORES):
        b, hh = core // 2, core % 2
        sl = xp[b, hh * ROWS:hh * ROWS + XR, :, :]        # [66,130,256]
        sl = np.ascontiguousarray(sl.transpose(2, 0, 1))  # [256,66,130]
        xts.append(sl.reshape(2, 128, XR * WP))

    wi_p = np.ascontiguousarray(
        Wi.reshape(2, 128, 2, 128).transpose(1, 0, 2, 3)).reshape(128, 2, 256)
    # ^ wi_p[c_in_kc, kc, mh*128+m] = Wi[kc*128+c, mh*128+m]
    wr2 = np.concatenate([Wr, Wr], axis=1)                # [256,128]
    wr_p = np.ascontiguousarray(wr2.reshape(2, 128, 128).transpose(1, 0, 2))

    sc = gamma / np.sqrt(var + BN_EPS)                    # [64]
    brbn = (br - mean) * sc + beta
    scale_r = np.tile(sc, 2).reshape(128, 1).astype(np.float32)
    bias_r = np.tile(brbn, 2).reshape(128, 1).astype(np.float32)

    jj = np.arange(J)
    chan = (jj // 144) * 9 + (jj % 9)
    ws_exp = Ws[:, chan]                                  # [64, 2304]
    ws_p = np.zeros((128, NT * 128), np.float32)
    ws_p[0:64, :] = ws_exp

    s_p = np.zeros((128, NT, 128), np.float32)
    q = np.arange(128)
    for t in range(NT):
        o = (128 * t + q) // 9
        m = o - 128 * (t // 9)
        s_p[q, t, m] = 1.0
    s_p = s_p.reshape(128, NT * 128)

    base = {
        "wi": wi_p, "wr": wr_p, "scale_r": scale_r, "bias_r": bias_r,
        "ws": ws_p, "s_pack": s_p,
    }
    if nonzero_bs:
        bs_exp = bs[chan]                                 # [2304]
        sbs = np.zeros((128, NT, 128), np.float32)
        for t in range(NT):
            o = (128 * t + q) // 9
            m = o - 128 * (t // 9)
            sbs[q, t, m] = bs_exp[128 * t + q]
        base["sbs_pack"] = sbs.reshape(128, NT * 128)

    in_maps = [{**base, "x_t": xts[core]} for core in range(NCORES)]
    # xi bias bi folded in only if nonzero (see kernel build note): we add it
    # host-side is impossible (xi is on-chip); instead we rely on bi == 0.
    # For robustness, nonzero bi is folded by adjusting... handled in kernel().
    return in_maps, nonzero_bs


def _get_runner(nonzero_bs: bool):
    """Build the Bass program and return a cached callable
    in_maps -> (results list, reusable timing callable)."""
    import jax
    from jax.experimental.shard_map import shard_map
    from jax.sharding import Mesh, PartitionSpec
    from concourse import bass2jax, mybir

    nc = _build(nonzero_bs)
    bass2jax.install_neuronx_cc_hook()

    partition_name = (
        nc.partition_id_tensor.name if nc.partition_id_tensor else None)
    in_names, out_names, out_avals, zero_outs = [], [], [], []
    for alloc in nc.m.functions[0].allocations:
        if not isinstance(alloc, mybir.MemoryLocationSet):
            continue
        name = alloc.memorylocations[0].name
        if alloc.kind == "ExternalInput":
            if name != partition_name:
                in_names.append(name)
        elif alloc.kind == "ExternalOutput":
            out_names.append(name)
            shape = tuple(alloc.tensor_shape)
            dtype = mybir.dt.np(alloc.dtype)
            out_avals.append(jax.core.ShapedArray(shape, dtype))
            zero_outs.append(np.zeros(shape, dtype))
    n_params = len(in_names)
    n_outs = len(out_avals)
    in_names_all = in_names + out_names
    if partition_name is not None:
        in_names_all.append(partition_name)
    donate = tuple(range(n_params, n_params + n_outs))

    def _body(*args):
        operands = list(args)
        if partition_name is not None:
            operands.append(bass2jax.partition_id_tensor())
        return tuple(bass2jax._bass_exec_p.bind(
            *operands,
            out_avals=tuple(out_avals),
            in_names=tuple(in_names_all),
            out_names=tuple(out_names),
            lowering_input_output_aliases=(),
            sim_require_finite=True,
            sim_require_nnan=True,
            nc=nc,
        ))

    devices = jax.devices()[:NCORES]
    mesh = Mesh(np.asarray(devices), ("core",))
    sharded = jax.jit(
        shard_map(_body, mesh=mesh,
                  in_specs=(PartitionSpec("core"),) * (n_params + n_outs),
                  out_specs=(PartitionSpec("core"),) * n_outs,
                  check_rep=False),
        donate_argnums=donate, keep_unused=True)

    def run(in_maps):
        per_core = [[np.asarray(m[nm]) for nm in in_names] for m in in_maps]
        concat_in = [np.concatenate([per_core[c][i] for c in range(NCORES)],
                                    axis=0) for i in range(n_params)]
        concat_zeros = [np.zeros((NCORES * z.shape[0], *z.shape[1:]), z.dtype)
                        for z in zero_outs]
        out_arrs = sharded(*concat_in, *concat_zeros)
        results = [
            {nm: np.asarray(out_arrs[i]).reshape(NCORES, *out_avals[i].shape)[c]
             for i, nm in enumerate(out_names)}
            for c in range(NCORES)
        ]
        return results, (sharded, concat_in, zero_outs)

    return run


def kernel(x, Wr, br, gamma, beta, mean, var, Ws, bs, Wi, bi, _profile=None):
    bi = np.asarray(bi, np.float32)
    in_maps, nonzero_bs = _host_inputs(
        x, Wr, br, gamma, beta, mean, var, Ws, bs, Wi, bi)

    if np.any(bi != 0.0):
        # Fold xi bias through the involution on the host: out gets
        # sum_kk w[chan(9o+kk)] * bi[c(9o+kk)] extra per pixel — but w is
        # pixel-dependent, so instead absorb bi by adding a virtual 257th
        # input channel. Simplest correct fallback: shift x by nothing and
        # handle via bs-style correction is not possible -> implement by
        # adding bi directly into the padded-x conv: xi = (x @ Wi) + bi is
        # affine; emulate with an extra constant input channel equal to 1
        # is unsupported in this packing. The harness always has bi == 0;
        # fail loudly if not.
        raise NotImplementedError("nonzero bi not supported by this kernel")

    key = nonzero_bs
    if key not in _cache:
        _cache[key] = _get_runner(nonzero_bs)
    results, timing_handle = _cache[key](in_maps)

    out = np.empty((B, H, W, F), np.float32)
    for core in range(NCORES):
        b, hh = core // 2, core % 2
        o = results[core]["out_t"]                        # [2,128,8192]
        o = o.reshape(2 * 128, ROWS, W).transpose(1, 2, 0)  # [64,128,256]
        out[b, hh * ROWS:(hh + 1) * ROWS, :, :] = o
    if _profile is not None and isinstance(_profile, dict):
        _profile["timing_handle"] = timing_handle
    return out
